# revision 54
# baseline (speedup 1.0000x reference)
"""Trainium2 Bass kernel for nn_Attention (dense transformer block).

Full-input contract: kernel(**inputs) takes the unsharded inputs and
returns the full output. 8 NeuronCores: tensor-parallel over head
groups (4 heads) x data-parallel over batch (2); core c = b*4 + g.
The per-core partial o_proj outputs are summed on the host (the
all-reduce of the row-sharded o_proj).

Per-core schedule (single pass over x^T): A(sc) projects V, Q, K for
one s-chunk (h-major groups accumulating in 4 PSUM banks). Attention
B(qc) (transposed-P flash style, no max-subtraction) interleaves with
filler = A(qc+1) + o_proj O(qc-2), deferred two chunks, so the PE
never waits on the softmax tail or exp latency. PV/den flushes lag
FLUSH_LAG blocks behind the score matmuls and head tails are shifted
after the next head's first block, giving each exp ~2-3us of slack.
Per-head softmax denominators accumulate in one PSUM bank at partition
offsets 0/32/64/96 (explicit matmul tile_position) so a single
fixed-cost vector Reciprocal serves all four heads per chunk; the
gpsimd broadcast reads physical partition 0, so rows are staged to a
partition-0 tile first. Banks: A b0-b3, scores b4/b5, ctx b6, den b7,
O b2/b3 (b0-b3 once the A stream is done). The sync engine issues DMA
descriptors serially (~0.65us each), so the prologue loads first-needed
tensors first (h-sliced) and xt prefetch fires a full phase early.

Matmul dtype fp16 (~8e-4 rel err). fp8 DoubleRow was measured at
exactly 2x fp16 per FLOP on HW, so error-compensated fp8 (3 DR matmuls
per 2 fp16) would be slower; plain fp8's ~3% error busts the gate.
"""
import contextlib
from collections import deque
import numpy as np
import concourse.bass as bass
from concourse import bacc
import concourse.mybir as mybir
import concourse.tile as tile
from concourse.bass_utils import run_bass_kernel_spmd

F32 = mybir.dt.float32
F32R = mybir.dt.float32r
F16 = mybir.dt.float16
BF16 = mybir.dt.bfloat16
EXP = mybir.ActivationFunctionType.Exp
LN = mybir.ActivationFunctionType.Ln
MMDT = {"f32r": F32R, "f16": F16, "bf16": BF16}

S = 2048
HID = 2048
D = 128
GH = 4            # heads per core
GW = GH * D       # 512
NCORES = 8
SC = S // 512     # 4 column chunks
HC = HID // 128   # 16 contraction chunks
SCALE = float(D) ** -0.5
NEG = -1.0e30

DTYPE = "f16"     # matmul dtype: 'f16' | 'bf16' | 'f32r'
FLUSH_LAG = 2


def _build(variant, dt):
    MDT = MMDT[dt]
    two_byte = dt in ("f16", "bf16")
    IDT = MDT if two_byte else F32
    nc = bacc.Bacc("TRN2", target_bir_lowering=False, debug=False,
                   num_devices=NCORES)
    xt = nc.dram_tensor("xt", [HID, S], IDT, kind="ExternalInput").ap()
    wq = nc.dram_tensor("wq", [HID, GW], IDT, kind="ExternalInput").ap()
    wk = nc.dram_tensor("wk", [HID, GW], IDT, kind="ExternalInput").ap()
    wv = nc.dram_tensor("wv", [HID, GW], IDT, kind="ExternalInput").ap()
    wo = nc.dram_tensor("wo", [GW, HID], IDT, kind="ExternalInput").ap()
    cost = nc.dram_tensor("cost", [D, S], IDT, kind="ExternalInput").ap()
    sint = nc.dram_tensor("sint", [D, S], IDT, kind="ExternalInput").ap()
    btpl = nc.dram_tensor("btpl", [D, 896], F32, kind="ExternalInput").ap()
    out = nc.dram_tensor("out", [S, HID], F32, kind="ExternalOutput").ap()

    def _bc(ap):
        return ap if two_byte else ap.bitcast(F32R)

    xt_r = _bc(xt.rearrange("(c p) s -> p c s", p=128))   # [128, 16, 2048]
    wq_r = _bc(wq.rearrange("(c p) m -> p c m", p=128))   # [128, 16, 512]
    wk_r = _bc(wk.rearrange("(c p) m -> p c m", p=128))
    wv_r = _bc(wv.rearrange("(c p) m -> p c m", p=128))
    wo_r = _bc(wo.rearrange("(c p) m -> p c m", p=128))   # [128, 4, 2048]

    XB = 4                   # h-chunks per xt DMA tile
    NXT = HC // XB           # 4 xt tiles per s-chunk

    with tile.TileContext(nc) as tc:
        with contextlib.ExitStack() as ctx:
            persist = ctx.enter_context(tc.tile_pool(name="persist", bufs=1))
            psum = ctx.enter_context(tc.tile_pool(name="psum", bufs=1, space="PSUM"))
            work = ctx.enter_context(tc.tile_pool(name="work", bufs=1))

            _n = [0]

            def bank(i, shape=(128, 512)):
                _n[0] += 1
                return psum.tile(list(shape), F32, tag=f"b{i}", name=f"bk{i}_{_n[0]}")

            qts = [[persist.tile([128, 512], MDT, tag=f"qt{h}_{s}",
                                 name=f"qt{h}_{s}") for s in range(SC)]
                   for h in range(GH)]
            kts = [[persist.tile([128, 512], MDT, tag=f"kt{h}_{s}",
                                 name=f"kt{h}_{s}") for s in range(SC)]
                   for h in range(GH)]
            vts = [persist.tile([128, GW], MDT, tag=f"v{st}", name=f"v{st}")
                   for st in range(HC)]
            cos_sb = persist.tile([128, S], MDT, tag="cos")
            sin_sb = persist.tile([128, S], MDT, tag="sin")
            btpl_sb = persist.tile([128, 896], F32, tag="btpl")
            ones_f = persist.tile([128, 1], F32, tag="onesf")
            ones = persist.tile([128, 1], MDT, tag="ones")
            wo_sb = persist.tile([128, GH, HID], MDT, tag="wo")
            wv_cs = [persist.tile([128, XB, GW], MDT, tag=f"wv{j}",
                                  name=f"wv{j}") for j in range(NXT)]
            wq_cs = [persist.tile([128, XB, GW], MDT, tag=f"wq{j}",
                                  name=f"wq{j}") for j in range(NXT)]
            wk_cs = [persist.tile([128, XB, GW], MDT, tag=f"wk{j}",
                                  name=f"wk{j}") for j in range(NXT)]

            def xt_tile(sc, j):
                t = work.tile([128, XB, 512], MDT, tag="xt", bufs=8,
                              name=f"xt_{sc}_{j}")
                nc.sync.dma_start(
                    out=t, in_=xt_r[:, j * XB:(j + 1) * XB,
                                    sc * 512:(sc + 1) * 512])
                return t

            # ---- prologue DMAs in need-order, h-sliced so the first
            # V-group's data lands with minimal serial-descriptor wait
            xts0 = []
            for j in range(NXT):
                for hh in range(XB):
                    nc.sync.dma_start(
                        out=wv_cs[j][:, hh:hh + 1, :],
                        in_=wv_r[:, j * XB + hh:j * XB + hh + 1, :])
                    if hh == 0:
                        t = work.tile([128, XB, 512], MDT, tag="xt", bufs=8,
                                      name=f"xt_0_{j}")
                        xts0.append(t)
                    nc.sync.dma_start(
                        out=xts0[j][:, hh:hh + 1, :],
                        in_=xt_r[:, j * XB + hh:j * XB + hh + 1, 0:512])
            for j in range(NXT):
                nc.sync.dma_start(out=wq_cs[j],
                                  in_=wq_r[:, j * XB:(j + 1) * XB, :])
            nc.sync.dma_start(out=cos_sb, in_=_bc(cost))
            nc.sync.dma_start(out=sin_sb, in_=_bc(sint))
            for j in range(NXT):
                nc.sync.dma_start(out=wk_cs[j],
                                  in_=wk_r[:, j * XB:(j + 1) * XB, :])
            nc.sync.dma_start(out=wo_sb, in_=wo_r)
            nc.sync.dma_start(out=btpl_sb, in_=btpl)
            nc.vector.memset(ones_f, 1.0)
            nc.vector.tensor_copy(ones, ones_f)

            # ---- A(sc): V, Q, K projections for one s-chunk ---------
            # h-major: each unit = 4 matmuls (one per bank) for one
            # (j, hh); 3 passes over xt (V to abanks, Q, K).
            # xt prefetch is issued a phase early via prep_unit(sc).
            xt_store = {0: xts0}

            def prep_unit(sc):
                def prep():
                    xt_store[sc] = [xt_tile(sc, j) for j in range(NXT)]
                return prep

            def a_units(sc, abanks=(0, 1, 2, 3), copy_evict=False):
                units = []
                state = {}
                nb = len(abanks)

                def bind():
                    state['xt'] = xt_store[sc]
                units.append(bind)

                ssl = slice(sc * 512, (sc + 1) * 512)

                def skew(mm_one, evict, nds):
                    # pipeline-skewed d-phases: lane n runs h-step k-n in
                    # unit k, so banks finish and evict one unit apart
                    # instead of all at the pass end
                    for k in range(HC + nds - 1):
                        def unit(k=k):
                            for n in range(nds):
                                h = k - n
                                if 0 <= h < HC:
                                    mm_one(n, h)
                        units.append(unit)
                        if k >= HC - 1:
                            units.append(lambda n=k - HC + 1, evict=evict:
                                         evict(n))

                def v_pass(sts):
                    ps = {}

                    def mm_one(n, h, sts=sts):
                        st = sts[n]
                        if h == 0:
                            ps[st] = bank(abanks[n % nb])
                        j, hh = divmod(h, XB)
                        nc.tensor.matmul(
                            ps[st],
                            state['xt'][j][:, hh, st * 128:(st + 1) * 128],
                            wv_cs[j][:, hh, :],
                            start=(h == 0), stop=(h == HC - 1))

                    def evict(n, sts=sts):
                        st = sts[n]
                        nc.scalar.copy(vts[sc * 4 + st], ps[st])

                    skew(mm_one, evict, len(sts))

                def qk_pass(w_cs, dsts, ds):
                    ps = {}

                    def mm_one(n, h, ds=ds):
                        d = ds[n]
                        if h == 0:
                            ps[d] = bank(abanks[n % nb])
                        j, hh = divmod(h, XB)
                        nc.tensor.matmul(
                            ps[d], w_cs[j][:, hh, d * 128:(d + 1) * 128],
                            state['xt'][j][:, hh, :],
                            start=(h == 0), stop=(h == HC - 1))

                    def evict(d):
                        # RoPE on DVE; copy_evict frees the bank fastest
                        # (matters only in the solo A(0) phase)
                        dst = dsts[d]
                        b = ps[d]
                        if copy_evict:
                            t2 = work.tile([128, 512], MDT, tag="t1", bufs=5,
                                           name=f"t2_{sc}_{dst.tensor.name}")
                            nc.vector.tensor_copy(t2, b)  # frees the bank
                            b = t2
                        t1 = work.tile([128, 512], MDT, tag="t1", bufs=5,
                                       name=f"t1_{sc}_{dst.tensor.name}")
                        nc.vector.tensor_mul(t1, b, cos_sb[:, ssl])
                        nc.vector.tensor_mul(dst[0:64, :], b[64:128, :],
                                             sin_sb[64:128, ssl])
                        nc.vector.tensor_mul(dst[64:128, :], b[0:64, :],
                                             sin_sb[0:64, ssl])  # frees bank
                        nc.vector.tensor_add(dst, dst, t1)

                    skew(mm_one, lambda n, ds=ds: evict(ds[n]), len(ds))

                if nb >= 4:
                    v_pass((0, 1, 2, 3))
                    qk_pass(wq_cs, [qts[d][sc] for d in range(GH)],
                            (0, 1, 2, 3))
                    qk_pass(wk_cs, [kts[d][sc] for d in range(GH)],
                            (0, 1, 2, 3))
                else:
                    v_pass((0, 1))
                    v_pass((2, 3))
                    qk_pass(wq_cs, [qts[d][sc] for d in range(GH)], (0, 1))
                    qk_pass(wq_cs, [qts[d][sc] for d in range(GH)], (2, 3))
                    qk_pass(wk_cs, [kts[d][sc] for d in range(GH)], (0, 1))
                    qk_pass(wk_cs, [kts[d][sc] for d in range(GH)], (2, 3))
                return units

            # ---- B(qc): attention only (no o_proj) ------------------
            def b_units(qc):
                if variant == "causal":
                    # old blocks first: their K/Q tiles are from earlier
                    # chunks, so the first scores never wait on the RoPE
                    # of the A pass that just finished
                    order = list(range(4 * qc)) + list(range(4 * qc, 4 * qc + 4))
                else:
                    order = list(range(HC))
                nkb = len(order)
                ctx_t = []
                head_kb = []
                head_tail = []
                qst = {'ctrs': []}
                for hd in range(GH):
                    st = {}

                    def start_head(st=st, hd=hd):
                        st['ctxps'] = bank(6)
                        if hd == 0:
                            qst['denbank'] = bank(7)
                        st['denps'] = qst['denbank'][32 * hd:32 * hd + 1, :]
                        st['pend'] = deque()
                        st['flushed'] = 0
                        st['dflushed'] = 0
                        st['prev'] = None

                    def flush(last, st=st, hd=hd):
                        pexp, kbp, lo = st['pend'].popleft()
                        first = st['flushed'] == 0
                        st['flushed'] += 1
                        nc.tensor.matmul(st['ctxps'][:, lo:],
                                         vts[kbp][:, hd * 128:(hd + 1) * 128],
                                         pexp[:, lo:], start=first, stop=last)
                        nc.tensor.matmul(st['denps'][:, lo:], ones,
                                         pexp[:, lo:],
                                         start=first, stop=last,
                                         skip_group_check=True,
                                         tile_position=(0, 32 * hd))

                    def kb_iter(i, kb, st=st, hd=hd, start_head=start_head,
                                flush=flush):
                        if i == 0:
                            start_head()
                        # diag block j: columns q_local < j*128 are fully
                        # masked — skip them in score/mask/exp/PV/den
                        diag = variant == "causal" and kb >= 4 * qc
                        lo = (kb - 4 * qc) * 128 if diag else 0
                        sps = bank(4 + i % 2)
                        nc.tensor.matmul(
                            sps[:, lo:],
                            kts[hd][kb // 4][:, (kb % 4) * 128:(kb % 4 + 1) * 128],
                            qts[hd][qc][:, lo:], start=True, stop=True)
                        if diag:
                            # triangle lives in the first 128 valid cols
                            nc.vector.tensor_add(sps[:, lo:lo + 128],
                                                 sps[:, lo:lo + 128],
                                                 btpl_sb[:, 384:512])
                        pexp = work.tile([128, 512], MDT, tag="pexp", bufs=5,
                                         name=f"pexp_{qc}_{hd}_{kb}")
                        nc.scalar.activation(pexp[:, lo:], sps[:, lo:],
                                             EXP, scale=SCALE)
                        st['pend'].append((pexp, kb, lo))
                        if len(st['pend']) > FLUSH_LAG:
                            flush(False)

                    def tail(st=st, hd=hd, flush=flush):
                        while len(st['pend']) > 1:
                            flush(False)
                        flush(True)
                        ctr = work.tile([128, 512], F32, tag="ctr", bufs=5,
                                        name=f"ctr_{qc}_{hd}")
                        nc.scalar.copy(ctr, st['ctxps'])  # frees ctx bank

                        def normalize(h2, ctr2, rcp_src):
                            # rcp_src: [1,512] reciprocal at partition 0
                            dbc = work.tile([128, 512], F32, tag="dbc",
                                            bufs=2, name=f"dbc_{qc}_{h2}")
                            nc.gpsimd.partition_broadcast(dbc, rcp_src)
                            ct = work.tile([128, 512], MDT, tag="ctx",
                                           bufs=12, name=f"ctx_{qc}_{h2}")
                            nc.vector.tensor_mul(ct, ctr2, dbc)
                            ctx_t.append(ct)

                        if True:
                            qst['ctrs'].append(ctr)
                            if hd == GH - 1:
                                # one batched reciprocal for all 4 heads
                                dsm = work.tile([128, 512], F32, tag="dsm",
                                                bufs=1, name=f"dsm_{qc}")
                                nc.vector.reciprocal(dsm, qst['denbank'])
                                for h2, ctr2 in enumerate(qst['ctrs']):
                                    # stage row to partition 0: the gpsimd
                                    # broadcast reads physical partition 0
                                    ds1 = work.tile([1, 512], F32, tag="ds1",
                                                    bufs=2,
                                                    name=f"ds1_{qc}_{h2}")
                                    nc.vector.tensor_copy(
                                        ds1, dsm[32 * h2:32 * h2 + 1, :])
                                    normalize(h2, ctr2, ds1)

                    head_kb.append([
                        (lambda i=i, kb=kb, kb_iter=kb_iter: kb_iter(i, kb))
                        for i, kb in enumerate(order)])
                    head_tail.append(tail)

                # stitch: tail of head h lands after head h+1's first kb
                units = list(head_kb[0])
                for hd in range(1, GH):
                    units.append(head_kb[hd][0])
                    units.append(head_tail[hd - 1])
                    units.extend(head_kb[hd][1:])
                units.append(head_tail[GH - 1])
                return units, ctx_t

            # ---- O(qc): o_proj --------------------------------------
            def o_units(qc, ctx_t, obanks=(2, 3)):
                units = []
                nb = len(obanks)
                pend_dma = deque()

                def flush_dma():
                    qb, ob, ot = pend_dma.popleft()
                    nc.sync.dma_start(
                        out=out[(qc * 4 + qb) * 128:
                                (qc * 4 + qb + 1) * 128,
                                ob * 512:(ob + 1) * 512],
                        in_=ot)

                for u in range(16):
                    qb, ob = divmod(u, 4)

                    def oproj(qb=qb, ob=ob, u=u):
                        ops = bank(obanks[u % nb])
                        for hd in range(GH):
                            nc.tensor.matmul(
                                ops, ctx_t[hd][:, qb * 128:(qb + 1) * 128],
                                wo_sb[:, hd, ob * 512:(ob + 1) * 512],
                                start=(hd == 0), stop=(hd == GH - 1))
                        ot = work.tile([128, 512], F32, tag="outsb", bufs=3,
                                       name=f"ot_{qc}_{qb}_{ob}")
                        if ob % 2 == 0:
                            nc.scalar.copy(ot, ops)
                        else:
                            nc.vector.tensor_copy(ot, ops)
                        # defer the out-DMA two units so the trigger never
                        # waits on the eviction copy in the issue queue
                        pend_dma.append((qb, ob, ot))
                        if len(pend_dma) > 2:
                            flush_dma()
                    units.append(oproj)

                def drain():
                    while pend_dma:
                        flush_dma()
                units.append(drain)
                return units

            def interleave(bu, filler):
                na, nb = len(filler), len(bu)
                ai = 0
                for i, u in enumerate(bu):
                    u()
                    tgt = (i + 1) * na // nb
                    while ai < tgt:
                        filler[ai]()
                        ai += 1
                while ai < na:
                    filler[ai]()
                    ai += 1

            # ---- emit ----------------------------------------------
            # xt(1) prefetch fires halfway through the solo A(0) phase;
            # xt(sc) for sc>=2 at the end of B(sc-2)'s filler, a full
            # phase before A(sc) runs.
            a0 = a_units(0, copy_evict=True)
            half = len(a0) // 2
            for u in a0[:half]:
                u()
            prep_unit(1)()
            for u in a0[half:]:
                u()
            ctxs = {}
            ou = {}
            for qc in range(SC):
                bu, ctx_t = b_units(qc)
                ctxs[qc] = ctx_t
                filler = []
                if qc + 2 < SC:
                    # xt prefetch up front: its ring slots freed a full
                    # phase ago, and late issue stalls the next A stream
                    filler += [prep_unit(qc + 2)]
                if qc + 1 < SC:
                    filler += a_units(qc + 1,
                                      abanks=(0, 1) if qc + 1 == SC - 1
                                      else (0, 1, 2, 3))
                post = []
                if qc == SC - 1:
                    # hold back the tail of O(qc-1) to cover the last
                    # chunk's softmax-tail chain before O(qc) can start
                    filler += ou[qc - 2] + ou[qc - 1][:8]
                    post = ou[qc - 1][8:]
                elif qc >= 2:
                    filler += ou[qc - 2]
                interleave(bu, filler)
                for u in post:
                    u()
                obanks = (0, 1, 2, 3) if qc >= SC - 2 else (2, 3)
                ou[qc] = o_units(qc, ctx_t, obanks=obanks)
            for u in ou[SC - 1]:
                u()
    nc.compile()
    return nc


_CACHE = {}


def _get(variant, dt=None):
    dt = dt or DTYPE
    if (variant, dt) not in _CACHE:
        _CACHE[(variant, dt)] = _build(variant, dt)
    return _CACHE[(variant, dt)]


def _rope_tables():
    inv = 1.0 / (10000.0 ** (np.arange(0, D, 2, dtype=np.float64) / D))  # [64]
    t = np.arange(S, dtype=np.float64)
    fr = np.outer(inv, t)                       # [64, S]
    cosT = np.concatenate([np.cos(fr), np.cos(fr)], 0).astype(np.float32)
    # partition-swapped sign-folded sin: rows 0:64 = +sin, rows 64:128 = -sin
    sinT = np.concatenate([np.sin(fr), -np.sin(fr)], 0).astype(np.float32)
    return cosT, sinT


def _btpl_causal():
    # additive mask template: NEG where k > c-384 else 0
    k = np.arange(128)[:, None]
    c = np.arange(896)[None, :]
    return np.where(k > c - 384, np.float32(NEG), np.float32(0.0)).astype(np.float32)


def _np_cast(a, dt):
    if dt == "f16":
        return a.astype(np.float16)
    if dt == "bf16":
        import ml_dtypes
        return a.astype(ml_dtypes.bfloat16)
    return a


def _numpy_fallback(hs, Wq, Wk, Wv, Wo, mask):
    B = hs.shape[0]
    cosT, sinT = _rope_tables()
    cos = cosT.T[None, :, None, :]
    sin = np.abs(sinT).T[None, :, None, :]
    outs = []
    for b in range(B):
        x = hs[b]
        q = (x @ Wq).reshape(S, 16, D)[None]
        k = (x @ Wk).reshape(S, 16, D)[None]
        vv = (x @ Wv).reshape(S, 16, D)

        def rope(z):
            z1, z2 = z[..., :64], z[..., 64:]
            rot = np.concatenate([-z2, z1], -1)
            return z * cos + rot * sin

        q, k = rope(q)[0], rope(k)[0]
        o = np.empty((S, 16, D), np.float32)
        m = mask[0, 0]
        for h in range(16):
            sc = (q[:, h] @ k[:, h].T) * SCALE
            sc = np.where(m == 0, -np.inf, sc)
            sc -= sc.max(-1, keepdims=True)
            p = np.exp(sc)
            p /= p.sum(-1, keepdims=True)
            o[:, h] = p @ vv[:, h]
        outs.append(o.reshape(S, HID) @ Wo)
    return np.stack(outs).astype(np.float32)


def build_in_maps(inputs):
    """Returns (in_maps, variant) or raises ValueError for fallback cases."""
    hs = np.asarray(inputs["hidden_states"], dtype=np.float32)
    Wq, Wk, Wv, Wo = (np.asarray(inputs[w], dtype=np.float32)
                      for w in ("Wq", "Wk", "Wv", "Wo"))
    mask = np.asarray(inputs["attention_mask"])
    m3 = mask.reshape(-1, mask.shape[-2], mask.shape[-1])
    m2 = m3[0]
    same = all(np.array_equal(m2, m3[i]) for i in range(1, m3.shape[0]))
    if not same:
        raise ValueError("per-batch masks")
    if np.all(m2 == 1):
        variant = "full"
    elif np.array_equal(m2 != 0, np.tril(np.ones((S, S), dtype=bool))):
        variant = "causal"
    else:
        raise ValueError("unsupported mask")

    cosT, sinT = _rope_tables()
    btpl = _btpl_causal() if variant == "causal" else np.zeros((128, 896), np.float32)

    in_maps = []
    for c in range(NCORES):
        b, g = divmod(c, GH)
        gsl = slice(g * GW, (g + 1) * GW)
        in_maps.append({
            "xt": _np_cast(np.ascontiguousarray(hs[b].T), DTYPE),
            "wq": _np_cast(np.ascontiguousarray(Wq[:, gsl]), DTYPE),
            "wk": _np_cast(np.ascontiguousarray(Wk[:, gsl]), DTYPE),
            "wv": _np_cast(np.ascontiguousarray(Wv[:, gsl]), DTYPE),
            "wo": _np_cast(np.ascontiguousarray(Wo[gsl, :]), DTYPE),
            "cost": _np_cast(cosT, DTYPE), "sint": _np_cast(sinT, DTYPE), "btpl": btpl,
        })
    return in_maps, variant


def kernel(hidden_states, Wq, Wk, Wv, Wo, attention_mask):
    hs = np.asarray(hidden_states, dtype=np.float32)
    Wq, Wk, Wv, Wo = (np.asarray(w, dtype=np.float32) for w in (Wq, Wk, Wv, Wo))
    mask = np.asarray(attention_mask)
    B = hs.shape[0]

    try:
        in_maps, variant = build_in_maps(dict(
            hidden_states=hs, Wq=Wq, Wk=Wk, Wv=Wv, Wo=Wo, attention_mask=mask))
    except ValueError:
        return _numpy_fallback(hs, Wq, Wk, Wv, Wo, mask)

    nc = _get(variant)
    res = run_bass_kernel_spmd(nc, in_maps, list(range(NCORES))).results
    out = np.zeros((B, S, HID), np.float32)
    for c in range(NCORES):
        b = c // GH
        out[b] += res[c]["out"]
    return out



# revision 55
# speedup vs baseline: 1.0221x; 1.0221x over previous
"""Trainium2 Bass kernel for nn_Attention (dense transformer block).

Full-input contract: kernel(**inputs) takes the unsharded inputs and
returns the full output. 8 NeuronCores: tensor-parallel over head
groups (4 heads) x data-parallel over batch (2); core c = b*4 + g.
The per-core partial o_proj outputs are summed on the host (the
all-reduce of the row-sharded o_proj).

Per-core schedule (single pass over x^T): A(sc) projects V, Q, K for
one s-chunk (h-major groups accumulating in 4 PSUM banks). Attention
B(qc) (transposed-P flash style, no max-subtraction) interleaves with
filler = A(qc+1) + o_proj O(qc-2), deferred two chunks, so the PE
never waits on the softmax tail or exp latency. PV/den flushes lag
FLUSH_LAG blocks behind the score matmuls and head tails are shifted
after the next head's first block, giving each exp ~2-3us of slack.
Per-head softmax denominators accumulate in one PSUM bank at partition
offsets 0/32/64/96 (explicit matmul tile_position) so a single
fixed-cost vector Reciprocal serves all four heads per chunk; the
gpsimd broadcast reads physical partition 0, so rows are staged to a
partition-0 tile first. Banks: A b0-b3, scores b4/b5, ctx b6, den b7,
O b2/b3 (b0-b3 once the A stream is done). The sync engine issues DMA
descriptors serially (~0.65us each), so the prologue loads first-needed
tensors first (h-sliced) and xt prefetch fires a full phase early.

Matmul dtype fp16 (~8e-4 rel err). fp8 DoubleRow was measured at
exactly 2x fp16 per FLOP on HW, so error-compensated fp8 (3 DR matmuls
per 2 fp16) would be slower; plain fp8's ~3% error busts the gate.
"""
import contextlib
from collections import deque
import numpy as np
import concourse.bass as bass
from concourse import bacc
import concourse.mybir as mybir
import concourse.tile as tile
from concourse.bass_utils import run_bass_kernel_spmd

F32 = mybir.dt.float32
F32R = mybir.dt.float32r
F16 = mybir.dt.float16
BF16 = mybir.dt.bfloat16
EXP = mybir.ActivationFunctionType.Exp
LN = mybir.ActivationFunctionType.Ln
MMDT = {"f32r": F32R, "f16": F16, "bf16": BF16}

S = 2048
HID = 2048
D = 128
GH = 4            # heads per core
GW = GH * D       # 512
NCORES = 8
SC = S // 512     # 4 column chunks
HC = HID // 128   # 16 contraction chunks
SCALE = float(D) ** -0.5
NEG = -1.0e30

DTYPE = "f16"     # matmul dtype: 'f16' | 'bf16' | 'f32r'
FLUSH_LAG = 2


def _build(variant, dt):
    MDT = MMDT[dt]
    two_byte = dt in ("f16", "bf16")
    IDT = MDT if two_byte else F32
    nc = bacc.Bacc("TRN2", target_bir_lowering=False, debug=False,
                   num_devices=NCORES)
    xt = nc.dram_tensor("xt", [HID, S], IDT, kind="ExternalInput").ap()
    wq = nc.dram_tensor("wq", [HID, GW], IDT, kind="ExternalInput").ap()
    wk = nc.dram_tensor("wk", [HID, GW], IDT, kind="ExternalInput").ap()
    wv = nc.dram_tensor("wv", [HID, GW], IDT, kind="ExternalInput").ap()
    wo = nc.dram_tensor("wo", [GW, HID], IDT, kind="ExternalInput").ap()
    cost = nc.dram_tensor("cost", [D, S], IDT, kind="ExternalInput").ap()
    sint = nc.dram_tensor("sint", [D, S], IDT, kind="ExternalInput").ap()
    btpl = nc.dram_tensor("btpl", [D, 896], F32, kind="ExternalInput").ap()
    out = nc.dram_tensor("out", [S, HID], F32, kind="ExternalOutput").ap()

    def _bc(ap):
        return ap if two_byte else ap.bitcast(F32R)

    xt_r = _bc(xt.rearrange("(c p) s -> p c s", p=128))   # [128, 16, 2048]
    wq_r = _bc(wq.rearrange("(c p) m -> p c m", p=128))   # [128, 16, 512]
    wk_r = _bc(wk.rearrange("(c p) m -> p c m", p=128))
    wv_r = _bc(wv.rearrange("(c p) m -> p c m", p=128))
    wo_r = _bc(wo.rearrange("(c p) m -> p c m", p=128))   # [128, 4, 2048]

    XB = 4                   # h-chunks per xt DMA tile
    NXT = HC // XB           # 4 xt tiles per s-chunk

    with tile.TileContext(nc) as tc:
        with contextlib.ExitStack() as ctx:
            persist = ctx.enter_context(tc.tile_pool(name="persist", bufs=1))
            psum = ctx.enter_context(tc.tile_pool(name="psum", bufs=1, space="PSUM"))
            work = ctx.enter_context(tc.tile_pool(name="work", bufs=1))

            _n = [0]

            def bank(i, shape=(128, 512)):
                _n[0] += 1
                return psum.tile(list(shape), F32, tag=f"b{i}", name=f"bk{i}_{_n[0]}")

            qts = [[persist.tile([128, 512], MDT, tag=f"qt{h}_{s}",
                                 name=f"qt{h}_{s}") for s in range(SC)]
                   for h in range(GH)]
            kts = [[persist.tile([128, 512], MDT, tag=f"kt{h}_{s}",
                                 name=f"kt{h}_{s}") for s in range(SC)]
                   for h in range(GH)]
            vts = [persist.tile([128, GW], MDT, tag=f"v{st}", name=f"v{st}")
                   for st in range(HC)]
            cos_sb = persist.tile([128, S], MDT, tag="cos")
            sin_sb = persist.tile([128, S], MDT, tag="sin")
            btpl_sb = persist.tile([128, 896], F32, tag="btpl")
            ones_f = persist.tile([128, 1], F32, tag="onesf")
            ones = persist.tile([128, 1], MDT, tag="ones")
            wo_sb = persist.tile([128, GH, HID], MDT, tag="wo")
            wv_cs = [persist.tile([128, XB, GW], MDT, tag=f"wv{j}",
                                  name=f"wv{j}") for j in range(NXT)]
            wq_cs = [persist.tile([128, XB, GW], MDT, tag=f"wq{j}",
                                  name=f"wq{j}") for j in range(NXT)]
            wk_cs = [persist.tile([128, XB, GW], MDT, tag=f"wk{j}",
                                  name=f"wk{j}") for j in range(NXT)]

            def xt_tile(sc, j):
                t = work.tile([128, XB, 512], MDT, tag="xt", bufs=8,
                              name=f"xt_{sc}_{j}")
                nc.sync.dma_start(
                    out=t, in_=xt_r[:, j * XB:(j + 1) * XB,
                                    sc * 512:(sc + 1) * 512])
                return t

            # ---- prologue DMAs in need-order, h-sliced so the first
            # V-group's data lands with minimal serial-descriptor wait
            xts0 = []
            for j in range(NXT):
                for hh in range(XB):
                    nc.sync.dma_start(
                        out=wv_cs[j][:, hh:hh + 1, :],
                        in_=wv_r[:, j * XB + hh:j * XB + hh + 1, :])
                    if hh == 0:
                        t = work.tile([128, XB, 512], MDT, tag="xt", bufs=8,
                                      name=f"xt_0_{j}")
                        xts0.append(t)
                    nc.sync.dma_start(
                        out=xts0[j][:, hh:hh + 1, :],
                        in_=xt_r[:, j * XB + hh:j * XB + hh + 1, 0:512])
            for j in range(NXT):
                nc.sync.dma_start(out=wq_cs[j],
                                  in_=wq_r[:, j * XB:(j + 1) * XB, :])
            nc.sync.dma_start(out=cos_sb, in_=_bc(cost))
            nc.sync.dma_start(out=sin_sb, in_=_bc(sint))
            for j in range(NXT):
                nc.sync.dma_start(out=wk_cs[j],
                                  in_=wk_r[:, j * XB:(j + 1) * XB, :])
            nc.sync.dma_start(out=wo_sb, in_=wo_r)
            nc.sync.dma_start(out=btpl_sb, in_=btpl)
            nc.vector.memset(ones_f, 1.0)
            nc.vector.tensor_copy(ones, ones_f)

            # ---- A(sc): V, Q, K projections for one s-chunk ---------
            # h-major: each unit = 4 matmuls (one per bank) for one
            # (j, hh); 3 passes over xt (V to abanks, Q, K).
            # xt prefetch is issued a phase early via prep_unit(sc).
            xt_store = {0: xts0}

            def prep_unit(sc):
                def prep():
                    xt_store[sc] = [xt_tile(sc, j) for j in range(NXT)]
                return prep

            def a_units(sc, abanks=(0, 1, 2, 3), copy_evict=False):
                units = []
                state = {}
                nb = len(abanks)

                def bind():
                    state['xt'] = xt_store[sc]
                units.append(bind)

                ssl = slice(sc * 512, (sc + 1) * 512)

                def skew(mm_one, evict, nds):
                    # pipeline-skewed d-phases: lane n runs h-step k-n in
                    # unit k, so banks finish and evict one unit apart
                    # instead of all at the pass end
                    for k in range(HC + nds - 1):
                        def unit(k=k):
                            for n in range(nds):
                                h = k - n
                                if 0 <= h < HC:
                                    mm_one(n, h)
                        units.append(unit)
                        if k >= HC - 1:
                            units.append(lambda n=k - HC + 1, evict=evict:
                                         evict(n))

                def v_pass(sts):
                    ps = {}

                    def mm_one(n, h, sts=sts):
                        st = sts[n]
                        if h == 0:
                            ps[st] = bank(abanks[n % nb])
                        j, hh = divmod(h, XB)
                        nc.tensor.matmul(
                            ps[st],
                            state['xt'][j][:, hh, st * 128:(st + 1) * 128],
                            wv_cs[j][:, hh, :],
                            start=(h == 0), stop=(h == HC - 1))

                    def evict(n, sts=sts):
                        st = sts[n]
                        nc.scalar.copy(vts[sc * 4 + st], ps[st])

                    skew(mm_one, evict, len(sts))

                def qk_pass(w_cs, dsts, ds):
                    ps = {}

                    def mm_one(n, h, ds=ds):
                        d = ds[n]
                        if h == 0:
                            ps[d] = bank(abanks[n % nb])
                        j, hh = divmod(h, XB)
                        nc.tensor.matmul(
                            ps[d], w_cs[j][:, hh, d * 128:(d + 1) * 128],
                            state['xt'][j][:, hh, :],
                            start=(h == 0), stop=(h == HC - 1))

                    def evict(d):
                        # RoPE on DVE; copy_evict frees the bank fastest
                        # (matters only in the solo A(0) phase)
                        dst = dsts[d]
                        b = ps[d]
                        if copy_evict:
                            t2 = work.tile([128, 512], MDT, tag="t1", bufs=5,
                                           name=f"t2_{sc}_{dst.tensor.name}")
                            nc.vector.tensor_copy(t2, b)  # frees the bank
                            b = t2
                        t1 = work.tile([128, 512], MDT, tag="t1", bufs=5,
                                       name=f"t1_{sc}_{dst.tensor.name}")
                        nc.vector.tensor_mul(t1, b, cos_sb[:, ssl])
                        nc.vector.tensor_mul(dst[0:64, :], b[64:128, :],
                                             sin_sb[64:128, ssl])
                        nc.vector.tensor_mul(dst[64:128, :], b[0:64, :],
                                             sin_sb[0:64, ssl])  # frees bank
                        nc.vector.tensor_add(dst, dst, t1)

                    skew(mm_one, lambda n, ds=ds: evict(ds[n]), len(ds))

                if nb >= 4:
                    v_pass((0, 1, 2, 3))
                    qk_pass(wq_cs, [qts[d][sc] for d in range(GH)],
                            (0, 1, 2, 3))
                    qk_pass(wk_cs, [kts[d][sc] for d in range(GH)],
                            (0, 1, 2, 3))
                else:
                    v_pass((0, 1))
                    v_pass((2, 3))
                    qk_pass(wq_cs, [qts[d][sc] for d in range(GH)], (0, 1))
                    qk_pass(wq_cs, [qts[d][sc] for d in range(GH)], (2, 3))
                    qk_pass(wk_cs, [kts[d][sc] for d in range(GH)], (0, 1))
                    qk_pass(wk_cs, [kts[d][sc] for d in range(GH)], (2, 3))
                return units

            # ---- B(qc): attention only (no o_proj) ------------------
            def b_units(qc):
                if variant == "causal":
                    # old blocks first: their K/Q tiles are from earlier
                    # chunks, so the first scores never wait on the RoPE
                    # of the A pass that just finished
                    order = list(range(4 * qc)) + list(range(4 * qc, 4 * qc + 4))
                else:
                    order = list(range(HC))
                nkb = len(order)
                ctx_t = []
                head_kb = []
                head_tail = []
                qst = {'ctrs': []}
                for hd in range(GH):
                    st = {}

                    def start_head(st=st, hd=hd):
                        st['ctxps'] = bank(6)
                        if hd == 0:
                            qst['denbank'] = bank(7)
                        st['denps'] = qst['denbank'][32 * hd:32 * hd + 1, :]
                        st['pend'] = deque()
                        st['flushed'] = 0
                        st['dflushed'] = 0
                        st['prev'] = None

                    def flush(last, st=st, hd=hd):
                        pexp, kbp, lo = st['pend'].popleft()
                        first = st['flushed'] == 0
                        st['flushed'] += 1
                        nc.tensor.matmul(st['ctxps'][:, lo:],
                                         vts[kbp][:, hd * 128:(hd + 1) * 128],
                                         pexp[:, lo:], start=first, stop=last)
                        nc.tensor.matmul(st['denps'][:, lo:], ones,
                                         pexp[:, lo:],
                                         start=first, stop=last,
                                         skip_group_check=True,
                                         tile_position=(0, 32 * hd))

                    def kb_iter(i, kb, st=st, hd=hd, start_head=start_head,
                                flush=flush):
                        if i == 0:
                            start_head()
                        # diag block j: columns q_local < j*128 are fully
                        # masked — skip them in score/mask/exp/PV/den
                        diag = variant == "causal" and kb >= 4 * qc
                        lo = (kb - 4 * qc) * 128 if diag else 0
                        sps = bank(4 + i % 2)
                        nc.tensor.matmul(
                            sps[:, lo:],
                            kts[hd][kb // 4][:, (kb % 4) * 128:(kb % 4 + 1) * 128],
                            qts[hd][qc][:, lo:], start=True, stop=True)
                        if diag:
                            # triangle lives in the first 128 valid cols
                            nc.vector.tensor_add(sps[:, lo:lo + 128],
                                                 sps[:, lo:lo + 128],
                                                 btpl_sb[:, 384:512])
                        pexp = work.tile([128, 512], MDT, tag="pexp", bufs=5,
                                         name=f"pexp_{qc}_{hd}_{kb}")
                        nc.scalar.activation(pexp[:, lo:], sps[:, lo:],
                                             EXP, scale=SCALE)
                        st['pend'].append((pexp, kb, lo))
                        if len(st['pend']) > FLUSH_LAG:
                            flush(False)

                    def tail(st=st, hd=hd, flush=flush):
                        while len(st['pend']) > 1:
                            flush(False)
                        flush(True)
                        ctr = work.tile([128, 512], F32, tag="ctr", bufs=5,
                                        name=f"ctr_{qc}_{hd}")
                        nc.scalar.copy(ctr, st['ctxps'])  # frees ctx bank

                        def normalize(h2, ctr2, rcp_src):
                            # rcp_src: [1,512] reciprocal at partition 0
                            dbc = work.tile([128, 512], F32, tag="dbc",
                                            bufs=2, name=f"dbc_{qc}_{h2}")
                            nc.gpsimd.partition_broadcast(dbc, rcp_src)
                            ct = work.tile([128, 512], MDT, tag="ctx",
                                           bufs=12, name=f"ctx_{qc}_{h2}")
                            nc.vector.tensor_mul(ct, ctr2, dbc)
                            ctx_t.append(ct)

                        if True:
                            qst['ctrs'].append(ctr)
                            if hd == GH - 1:
                                # one batched reciprocal for all 4 heads
                                dsm = work.tile([128, 512], F32, tag="dsm",
                                                bufs=1, name=f"dsm_{qc}")
                                nc.vector.reciprocal(dsm, qst['denbank'])
                                for h2, ctr2 in enumerate(qst['ctrs']):
                                    # stage row to partition 0: the gpsimd
                                    # broadcast reads physical partition 0
                                    ds1 = work.tile([1, 512], F32, tag="ds1",
                                                    bufs=2,
                                                    name=f"ds1_{qc}_{h2}")
                                    nc.vector.tensor_copy(
                                        ds1, dsm[32 * h2:32 * h2 + 1, :])
                                    normalize(h2, ctr2, ds1)

                    head_kb.append([
                        (lambda i=i, kb=kb, kb_iter=kb_iter: kb_iter(i, kb))
                        for i, kb in enumerate(order)])
                    head_tail.append(tail)

                # stitch: tail of head h lands after head h+1's first kb
                units = list(head_kb[0])
                for hd in range(1, GH):
                    units.append(head_kb[hd][0])
                    units.append(head_tail[hd - 1])
                    units.extend(head_kb[hd][1:])
                units.append(head_tail[GH - 1])
                return units, ctx_t

            # ---- O(qc): o_proj --------------------------------------
            def o_units(qc, ctx_t, obanks=(2, 3)):
                units = []
                nb = len(obanks)
                for u in range(16):
                    qb, ob = divmod(u, 4)

                    def oproj(qb=qb, ob=ob, u=u):
                        ops = bank(obanks[u % nb])
                        for hd in range(GH):
                            nc.tensor.matmul(
                                ops, ctx_t[hd][:, qb * 128:(qb + 1) * 128],
                                wo_sb[:, hd, ob * 512:(ob + 1) * 512],
                                start=(hd == 0), stop=(hd == GH - 1))
                        ot = work.tile([128, 512], F32, tag="outsb", bufs=3,
                                       name=f"ot_{qc}_{qb}_{ob}")
                        if ob % 2 == 0:
                            nc.scalar.copy(ot, ops)
                        else:
                            nc.vector.tensor_copy(ot, ops)
                        nc.sync.dma_start(
                            out=out[(qc * 4 + qb) * 128:
                                    (qc * 4 + qb + 1) * 128,
                                    ob * 512:(ob + 1) * 512],
                            in_=ot)
                    units.append(oproj)
                return units

            def interleave(bu, filler):
                na, nb = len(filler), len(bu)
                ai = 0
                for i, u in enumerate(bu):
                    u()
                    tgt = (i + 1) * na // nb
                    while ai < tgt:
                        filler[ai]()
                        ai += 1
                while ai < na:
                    filler[ai]()
                    ai += 1

            # ---- emit ----------------------------------------------
            # xt(1) prefetch fires halfway through the solo A(0) phase;
            # xt(sc) for sc>=2 at the end of B(sc-2)'s filler, a full
            # phase before A(sc) runs.
            a0 = a_units(0, copy_evict=True)
            half = len(a0) // 2
            for u in a0[:half]:
                u()
            prep_unit(1)()
            for u in a0[half:]:
                u()
            ctxs = {}
            ou = {}
            for qc in range(SC):
                bu, ctx_t = b_units(qc)
                ctxs[qc] = ctx_t
                filler = []
                if qc + 2 < SC:
                    # xt prefetch up front: its ring slots freed a full
                    # phase ago, and late issue stalls the next A stream
                    filler += [prep_unit(qc + 2)]
                if qc + 1 < SC:
                    filler += a_units(qc + 1,
                                      abanks=(0, 1) if qc + 1 == SC - 1
                                      else (0, 1, 2, 3))
                post = []
                if qc == SC - 1:
                    # hold back the tail of O(qc-1) to cover the last
                    # chunk's softmax-tail chain before O(qc) can start
                    filler += ou[qc - 2] + ou[qc - 1][:8]
                    post = ou[qc - 1][8:]
                elif qc >= 2:
                    filler += ou[qc - 2]
                interleave(bu, filler)
                for u in post:
                    u()
                obanks = (0, 1, 2, 3) if qc >= SC - 2 else (2, 3)
                ou[qc] = o_units(qc, ctx_t, obanks=obanks)
            for u in ou[SC - 1]:
                u()
    nc.compile()
    return nc


_CACHE = {}


def _get(variant, dt=None):
    dt = dt or DTYPE
    if (variant, dt) not in _CACHE:
        _CACHE[(variant, dt)] = _build(variant, dt)
    return _CACHE[(variant, dt)]


def _rope_tables():
    inv = 1.0 / (10000.0 ** (np.arange(0, D, 2, dtype=np.float64) / D))  # [64]
    t = np.arange(S, dtype=np.float64)
    fr = np.outer(inv, t)                       # [64, S]
    cosT = np.concatenate([np.cos(fr), np.cos(fr)], 0).astype(np.float32)
    # partition-swapped sign-folded sin: rows 0:64 = +sin, rows 64:128 = -sin
    sinT = np.concatenate([np.sin(fr), -np.sin(fr)], 0).astype(np.float32)
    return cosT, sinT


def _btpl_causal():
    # additive mask template: NEG where k > c-384 else 0
    k = np.arange(128)[:, None]
    c = np.arange(896)[None, :]
    return np.where(k > c - 384, np.float32(NEG), np.float32(0.0)).astype(np.float32)


def _np_cast(a, dt):
    if dt == "f16":
        return a.astype(np.float16)
    if dt == "bf16":
        import ml_dtypes
        return a.astype(ml_dtypes.bfloat16)
    return a


def _numpy_fallback(hs, Wq, Wk, Wv, Wo, mask):
    B = hs.shape[0]
    cosT, sinT = _rope_tables()
    cos = cosT.T[None, :, None, :]
    sin = np.abs(sinT).T[None, :, None, :]
    outs = []
    for b in range(B):
        x = hs[b]
        q = (x @ Wq).reshape(S, 16, D)[None]
        k = (x @ Wk).reshape(S, 16, D)[None]
        vv = (x @ Wv).reshape(S, 16, D)

        def rope(z):
            z1, z2 = z[..., :64], z[..., 64:]
            rot = np.concatenate([-z2, z1], -1)
            return z * cos + rot * sin

        q, k = rope(q)[0], rope(k)[0]
        o = np.empty((S, 16, D), np.float32)
        m = mask[0, 0]
        for h in range(16):
            sc = (q[:, h] @ k[:, h].T) * SCALE
            sc = np.where(m == 0, -np.inf, sc)
            sc -= sc.max(-1, keepdims=True)
            p = np.exp(sc)
            p /= p.sum(-1, keepdims=True)
            o[:, h] = p @ vv[:, h]
        outs.append(o.reshape(S, HID) @ Wo)
    return np.stack(outs).astype(np.float32)


def build_in_maps(inputs):
    """Returns (in_maps, variant) or raises ValueError for fallback cases."""
    hs = np.asarray(inputs["hidden_states"], dtype=np.float32)
    Wq, Wk, Wv, Wo = (np.asarray(inputs[w], dtype=np.float32)
                      for w in ("Wq", "Wk", "Wv", "Wo"))
    mask = np.asarray(inputs["attention_mask"])
    m3 = mask.reshape(-1, mask.shape[-2], mask.shape[-1])
    m2 = m3[0]
    same = all(np.array_equal(m2, m3[i]) for i in range(1, m3.shape[0]))
    if not same:
        raise ValueError("per-batch masks")
    if np.all(m2 == 1):
        variant = "full"
    elif np.array_equal(m2 != 0, np.tril(np.ones((S, S), dtype=bool))):
        variant = "causal"
    else:
        raise ValueError("unsupported mask")

    cosT, sinT = _rope_tables()
    btpl = _btpl_causal() if variant == "causal" else np.zeros((128, 896), np.float32)

    in_maps = []
    for c in range(NCORES):
        b, g = divmod(c, GH)
        gsl = slice(g * GW, (g + 1) * GW)
        in_maps.append({
            "xt": _np_cast(np.ascontiguousarray(hs[b].T), DTYPE),
            "wq": _np_cast(np.ascontiguousarray(Wq[:, gsl]), DTYPE),
            "wk": _np_cast(np.ascontiguousarray(Wk[:, gsl]), DTYPE),
            "wv": _np_cast(np.ascontiguousarray(Wv[:, gsl]), DTYPE),
            "wo": _np_cast(np.ascontiguousarray(Wo[gsl, :]), DTYPE),
            "cost": _np_cast(cosT, DTYPE), "sint": _np_cast(sinT, DTYPE), "btpl": btpl,
        })
    return in_maps, variant


def kernel(hidden_states, Wq, Wk, Wv, Wo, attention_mask):
    hs = np.asarray(hidden_states, dtype=np.float32)
    Wq, Wk, Wv, Wo = (np.asarray(w, dtype=np.float32) for w in (Wq, Wk, Wv, Wo))
    mask = np.asarray(attention_mask)
    B = hs.shape[0]

    try:
        in_maps, variant = build_in_maps(dict(
            hidden_states=hs, Wq=Wq, Wk=Wk, Wv=Wv, Wo=Wo, attention_mask=mask))
    except ValueError:
        return _numpy_fallback(hs, Wq, Wk, Wv, Wo, mask)

    nc = _get(variant)
    res = run_bass_kernel_spmd(nc, in_maps, list(range(NCORES))).results
    out = np.zeros((B, S, HID), np.float32)
    for c in range(NCORES):
        b = c // GH
        out[b] += res[c]["out"]
    return out

